# revision 58
# baseline (speedup 1.0000x reference)
"""Trainium2 Bass kernel for a GQA attention layer (dense transformer).

Reference computation (B=1, S=2048, DIM=2048, 32 q-heads, 8 kv-heads, hd=64):
    xq = x @ wq; xk = x @ wk; xv = x @ wv
    rope(xq, xk); GQA causal attention; out = attn @ wo

Sharding: tensor-parallel over heads across 8 cores. Core c owns q-heads
4c..4c+3 (wq cols), kv-head c (wk/wv cols), and wo rows 256c..256c+255.
Each core computes a full [S, DIM] partial of the output projection; the
host sums the 8 partials (the TP all-reduce, done at gather time).

fp8 strategy (e4m3 + DoubleRow = 0.5 PE-cycles/row over 2x128 contraction):
  - Projections: split-3 hi/lo fp8 (x = x8+xlo, w = w8+wlo, host-prepped,
    pow2-scaled).  x8@w8 + xlo@w8 per k-slice packed in one DoubleRow
    (x-side exact); x8@wlo over k-slice pairs.  0.75x of bf16 PE cost,
    error ~0.1%.
  - Scores: one DoubleRow per tile with slices (K8*Q8 + Klo*Q8): K-side
    bf16-exact (hi/lo planes), Q-side plain e4m3 with a stride-0 broadcast
    AP as the moving tensor (~1.6% rel err, inside the 2e-2 budget).
    0.5x of bf16 PE cost.
  - PV / WO stay bf16 (P residuals are unaffordable on-chip).

Kernel layout strategy (everything "transposed", head_dim on partitions):
  - RoPE pairs de-interleaved by permuting wq/wk columns on the host so the
    rotation partner sits 16 partitions away; pow2 scales folded into the
    cos/sin constants.
  - Scores computed transposed: S^T[k, q] per 128-row k-tile; exp on ACT
    (scale fused); causal mask = upper-tri multiply on diagonal blocks.
  - P@V as V'.T @ P^T with V' = [V | ones] (row 64 = softmax denominator).
  - Normalization: reciprocal (DVE) + partition_broadcast (GPSIMD) + mult.
  - RoPE inputs pulled PSUM->SBUF bf16 by ACT (activation Copy, unscaling
    fused) so the DVE rope ops run in 2-byte all-SBUF fast modes; causal
    masking runs on GPSIMD as an affine_select predicate.
  - Output projection from O^T with wo shard as rhs; partial DMA'd bf16
    (summed f32 on the host).
"""

import numpy as np
import ml_dtypes

import concourse.bass as bass
import concourse.mybir as mybir
from concourse import bacc
from concourse.tile import TileContext
from concourse.masks import make_identity
from concourse.bass_utils import run_bass_kernel_spmd

# ---------------------------------------------------------------- constants
S = 2048          # sequence length
DIM = 2048        # model dim
NH = 32           # query heads
NKV = 8           # kv heads
HD = 64           # head dim
NCORES = 8
HQ = NH // NCORES          # query heads per core = 4
QW = HQ * HD               # q width per core = 256
KT_S = S // 128            # 16 seq k-tiles
KT_D = DIM // 128          # 16 dim k-tiles
NSC = S // 512             # 4 s-chunks
SCALE = 1.0 / 8.0          # 1/sqrt(64)

SX = 16.0                  # x fp8 scale
SW = 1024.0                # wq/wk/wv fp8 scale
SQ = 16.0                  # q/k fp8 scale (shared so one cos/sin set works)

WQKV = QW + 2 * HD  # 384

_SHUF_SWAP16 = list(range(16, 32)) + list(range(16))

F8 = mybir.dt.float8e4
BF16 = mybir.dt.bfloat16
F32 = mybir.dt.float32
DR = mybir.MatmulPerfMode.DoubleRow
EXP = mybir.ActivationFunctionType.Exp
MULT = mybir.AluOpType.mult
ADD = mybir.AluOpType.add


def build_program():
    """Build the per-core Bass program (same program on all 8 cores).

    Emission is a fine-grained software pipeline: attention beats for chunk
    sc (S^T matmuls for head h + PV pairs of head h-1) are merged with the
    projection matmuls of chunk sc+1 and the WO units of chunk sc-1.

    PSUM (8 banks): pjo 3 (projection passes + PV accumulators, shared tag)
    + ps 4 (two [128,1024] score megas) + pw 1 (WO + V/warm transposes).
    """
    nc = bacc.Bacc("TRN2", target_bir_lowering=False, debug=False,
                   num_devices=NCORES)

    xT8 = nc.dram_tensor("xT8", [DIM, S], F8, kind="ExternalInput")
    xTlo = nc.dram_tensor("xTlo", [DIM, S], F8, kind="ExternalInput")
    wqkv8 = nc.dram_tensor("wqkv8", [DIM, WQKV], F8, kind="ExternalInput")
    wqkvlo = nc.dram_tensor("wqkvlo", [DIM, WQKV], F8, kind="ExternalInput")
    wo_s = nc.dram_tensor("wo_s", [QW, DIM], BF16, kind="ExternalInput")
    cosE = nc.dram_tensor("cosE", [64, S], BF16, kind="ExternalInput")
    sinE = nc.dram_tensor("sinE", [64, S], BF16, kind="ExternalInput")
    out = nc.dram_tensor("out", [S, DIM], BF16, kind="ExternalOutput")

    import contextlib
    with TileContext(nc) as tc, contextlib.ExitStack() as ctx:
        const = ctx.enter_context(tc.tile_pool(name="const", bufs=1))
        work = ctx.enter_context(tc.tile_pool(name="work", bufs=3))
        xtp = ctx.enter_context(tc.tile_pool(name="xtp", bufs=8))
        ptp = ctx.enter_context(tc.tile_pool(name="ptp", bufs=21))
        small = ctx.enter_context(tc.tile_pool(name="small", bufs=7))
        osb = ctx.enter_context(tc.tile_pool(name="osb", bufs=6))

        pjo = ctx.enter_context(tc.tile_pool(name="pjo", bufs=3,
                                             space="PSUM"))
        ps = ctx.enter_context(tc.tile_pool(name="ps", bufs=2, space="PSUM"))
        pw = ctx.enter_context(tc.tile_pool(name="pw", bufs=1, space="PSUM"))

        # ----------------------------------------------- persistent SBUF
        w8 = const.tile([128, KT_D, WQKV], F8, tag="w8")
        wlo = const.tile([128, KT_D, WQKV], F8, tag="wlo")
        wo_sb = const.tile([128, 2 * DIM], BF16, tag="wo_sb")
        cos_sb = const.tile([128, S], BF16, tag="cos_sb")
        sin_sb = const.tile([128, S], BF16, tag="sin_sb")
        ident = const.tile([128, 128], BF16, tag="ident")
        Q8 = const.tile([64, HQ * S], F8, tag="Q8")
        # K planes: [:,0,:] = K8 (hi), [:,1,:] = Klo (residual): the
        # DoubleRow lhsT slices make the K side bf16-exact; Q is plain e4m3
        K8 = const.tile([64, 2, S], F8, tag="K8")
        Vt = const.tile([128, S], BF16, tag="Vt")  # V in rows 64:128
        Vp = const.tile([128, KT_S * (HD + 1)], BF16, tag="Vp")
        OT = const.tile([128, 2 * S], BF16, tag="OT")

        make_identity(nc, ident[:])
        nc.gpsimd.memset(Vp[:], 1.0)  # ones columns for denominator

        # PE warm-up: burn the pstate ramp on dependency-free transposes
        # while the first weight/x DMAs are in flight (each real matmul in
        # the first 3us of a busy window runs at half speed otherwise)
        warm = pw.tile([128, 128], BF16, tag="pw", name="warm")
        for _ in range(24):
            nc.tensor.transpose(warm[:], ident[:], ident[:])

        wo_copy_flip = [0]

        # ---------------------------------------------- thunk generators
        def proj_thunks(sc, fused=False):
            """Projection of chunk sc.

            Three DoubleRow passes per output slab, each packing k-slice
            pairs in the two DR slices: x8@w8, xlo@w8, x8@wlo.  All
            accumulate one PSUM tile per slab (KV / Q0 / Q1)."""
            s0 = sc * 512
            xts = []
            st = {}

            batches = [4, 4, 4, 4]
            starts = [sum(batches[:i]) for i in range(len(batches))]
            kt_slot = {}
            for bi, (b0, bn) in enumerate(zip(starts, batches)):
                for j in range(bn):
                    kt_slot[b0 + j] = (bi, j, bn, b0)

            def slab_off(slab):
                # slab 0,1 = Q cols [0:128],[128:256]; slab 2 = KV [256:384]
                return 256 if slab == 2 else slab * 128

            def drp(slab, pi, term):
                # k-slice pair (2pi, 2pi+1); term 0: x8@w8, 1: xlo@w8,
                # 2: x8@wlo
                kt0 = 2 * pi
                bi, j, bn, b0 = kt_slot[kt0]
                assert kt_slot[kt0 + 1][0] == bi  # same batch tile
                xt = xts[bi]
                off = slab_off(slab)
                wt = wlo if term == 2 else w8
                xplane = 1 if term == 1 else 0
                nc.tensor.matmul(
                    st[f"p{slab}"][:],
                    wt[:, kt0:kt0 + 2, off:off + 128],
                    xt[:, xplane, j * 512:j * 512 + 1024].rearrange(
                        "p (k f) -> p k f", k=2),
                    start=(pi == 0 and term == 0),
                    stop=(pi == KT_D // 2 - 1 and term == 2),
                    perf_mode=DR)

            def dma_kv(kt):
                bi, j, bn, b0 = kt_slot[kt]
                if j == 0:
                    if sc == 0:
                        nc.sync.dma_start(
                            w8[:, b0:b0 + bn, :],
                            wqkv8[b0 * 128:(b0 + bn) * 128, :].rearrange(
                                "(k r) w -> r k w", k=bn))
                        nc.sync.dma_start(
                            wlo[:, b0:b0 + bn, :],
                            wqkvlo[b0 * 128:(b0 + bn) * 128, :].rearrange(
                                "(k r) w -> r k w", k=bn))
                    xt4 = xtp.tile([128, 2, 4 * 512], F8, tag="xt",
                                   name="xt4")
                    nc.sync.dma_start(
                        xt4[:, 0, 0:bn * 512].rearrange(
                            "r (k c) -> r k c", k=bn),
                        xT8[b0 * 128:(b0 + bn) * 128,
                            s0:s0 + 512].rearrange("(k r) c -> r k c", k=bn))
                    nc.sync.dma_start(
                        xt4[:, 1, 0:bn * 512].rearrange(
                            "r (k c) -> r k c", k=bn),
                        xTlo[b0 * 128:(b0 + bn) * 128,
                             s0:s0 + 512].rearrange("(k r) c -> r k c", k=bn))
                    xts.append(xt4)
                    if kt == (4 if sc == 0 else 0):
                        # stream only this chunk's cos/sin columns
                        for tsb, tdr in ((cos_sb, cosE), (sin_sb, sinE)):
                            nc.sync.dma_start(
                                tsb[0:64, s0:s0 + 512],
                                tdr[:, s0:s0 + 512])
                            nc.sync.dma_start(
                                tsb[64:128, s0:s0 + 512],
                                tdr[:, s0:s0 + 512])
                if kt == 0:
                    st["p2"] = pjo.tile([128, 512], F32, tag="pjo",
                                        name="pkv")
                    if fused:
                        st["p0"] = pjo.tile([128, 512], F32, tag="pjo",
                                            name="fq0")
                        st["p1"] = pjo.tile([128, 512], F32, tag="pjo",
                                            name="fq1")
                if kt % 2 == 1:
                    for term in range(3):
                        drp(2, kt // 2, term)
                        if fused:
                            for mt in range(2):
                                drp(mt, kt // 2, term)
                if fused and kt % 4 == 3 and kt < 15:
                    # prologue is DMA-paced: each batch's DRs underfill the
                    # PE vs its ~3us DMA; top up with free transposes so the
                    # engine neither idles nor cools down
                    for _ in range(10):
                        nc.tensor.transpose(warm[:], ident[:], ident[:])

            def k_rope():
                pkv = st["p2"]
                # ACT pulls the PSUM slab to SBUF bf16 (unscaling sx*sw) so
                # every DVE op below runs in 2-byte all-SBUF 4x mode
                ks = work.tile([128, 512], BF16, tag="ks", name="ks")
                nc.scalar.activation(ks[:], pkv[:],
                                     mybir.ActivationFunctionType.Copy,
                                     scale=1.0 / (SX * SW))
                shufk = work.tile([64, 512], BF16, tag="shufk", name="shufk")
                m1k = work.tile([64, 512], BF16, tag="m1k", name="m1k")
                t2k = work.tile([64, 512], BF16, tag="t2k", name="t2k")
                kb = work.tile([64, 512], BF16, tag="kb", name="kb")
                nc.vector.stream_shuffle(shufk[:], ks[0:64, :],
                                         _SHUF_SWAP16)
                nc.vector.tensor_mul(m1k[:], ks[0:64, :],
                                     cos_sb[0:64, s0:s0 + 512])
                nc.vector.tensor_mul(t2k[:], shufk[:],
                                     sin_sb[0:64, s0:s0 + 512])
                nc.vector.tensor_add(kb[:], m1k[:], t2k[:])
                nc.vector.tensor_copy(K8[:, 0, s0:s0 + 512], kb[:])
                nc.vector.scalar_tensor_tensor(
                    K8[:, 1, s0:s0 + 512], K8[:, 0, s0:s0 + 512], -1.0,
                    kb[:], op0=MULT, op1=ADD)
                nc.vector.tensor_copy(Vt[64:128, s0:s0 + 512],
                                      ks[64:128, :])

            def v_trans(kt):
                pv = pw.tile([128, HD], BF16, tag="pw", name="pv")
                nc.tensor.transpose(
                    pv[:], Vt[64:128, kt * 128:(kt + 1) * 128],
                    ident[64:128, 64:128])
                nc.vector.tensor_copy(
                    Vp[:, kt * (HD + 1):kt * (HD + 1) + HD], pv[:])

            def q_mm(mt, kt):
                if kt == 0:
                    st[f"p{mt}"] = pjo.tile([128, 512], F32, tag="pjo",
                                            name="pq")
                if kt % 2 == 1:
                    for term in range(3):
                        drp(mt, kt // 2, term)

            def q_rope(mt):
                pq = st[f"p{mt}"]
                qs = work.tile([128, 512], BF16, tag="qs", name="qs")
                nc.scalar.activation(qs[:], pq[:],
                                     mybir.ActivationFunctionType.Copy,
                                     scale=1.0 / (SX * SW))
                shuf = work.tile([128, 512], BF16, tag="shuf", name="shuf")
                m1 = work.tile([128, 512], BF16, tag="m1", name="m1")
                t2 = work.tile([128, 512], BF16, tag="t2", name="t2")
                nc.vector.stream_shuffle(shuf[:], qs[:], _SHUF_SWAP16)
                nc.vector.tensor_mul(m1[:], qs[:], cos_sb[:, s0:s0 + 512])
                nc.vector.tensor_mul(t2[:], shuf[:], sin_sb[:, s0:s0 + 512])
                he = (2 * mt) * S
                ho = (2 * mt + 1) * S
                # fp8 write directly from the rope add (plain-e4m3 Q side)
                nc.vector.tensor_add(Q8[:, he + s0:he + s0 + 512],
                                     m1[0:64, :], t2[0:64, :])
                nc.vector.tensor_add(Q8[:, ho + s0:ho + s0 + 512],
                                     m1[64:128, :], t2[64:128, :])

            th = [lambda kt=kt: dma_kv(kt) for kt in range(KT_D)]
            if fused:
                th += [lambda: q_rope(0), k_rope, lambda: q_rope(1)]
                th += [lambda kt=kt: v_trans(kt)
                       for kt in range(4 * sc, 4 * sc + 4)]
                return th
            th.append(k_rope)
            th += [lambda kt=kt: v_trans(kt)
                   for kt in range(4 * sc, 4 * sc + 4)]
            if True:
                for mt in range(2):
                    th += [lambda mt=mt, kt=kt: q_mm(mt, kt)
                           for kt in range(KT_D)]
                    th.append(lambda mt=mt: q_rope(mt))
            return th

        def s_thunks(qc, h, tiles):
            """S^T DoubleRow matmuls + exp + mask for one head."""
            q0 = qc * 512
            hf = h * S
            nkt = 4 * qc + 4
            thunks = []
            for pi in range(nkt // 2):
                def th(pi=pi):
                    kts = (2 * pi, 2 * pi + 1)
                    ps_t = ps.tile([128, 1024], F32, tag="ps", name="ps_t")
                    pt_t = ptp.tile([128, 1024], BF16, tag="pt", name="pt_t")
                    for li, kt in enumerate(kts):
                        dj = kt - 4 * qc
                        qo = 128 * dj if dj >= 0 else 0
                        if 2 * pi == 4 * qc:
                            qo = 0  # keep the merged full-width exp legal
                        lo = li * 512
                        nc.tensor.matmul(
                            ps_t[:, lo + qo:lo + 512],
                            K8[:, :, kt * 128:(kt + 1) * 128],
                            Q8[:, hf + q0 + qo:hf + q0 + 512]
                            .unsqueeze(1).broadcast_to((64, 2, 512 - qo)),
                            start=True, stop=True, perf_mode=DR)
                    if 2 * pi + 1 < 4 * qc or 2 * pi == 4 * qc:
                        # interior pair, or the (dj0,dj1) boundary pair:
                        # one full-width exp (the dj1 junk region is never
                        # read downstream) — fewer ACT instructions
                        nc.scalar.activation(
                            pt_t[:], ps_t[:], EXP,
                            scale=SCALE / (SQ * SQ))
                    else:
                        for li, kt in enumerate(kts):
                            dj = kt - 4 * qc
                            qo = 128 * dj if dj >= 0 else 0
                            lo = li * 512
                            nc.scalar.activation(
                                pt_t[:, lo + qo:lo + 512],
                                ps_t[:, lo + qo:lo + 512], EXP,
                                scale=SCALE / (SQ * SQ))
                    for li, kt in enumerate(kts):
                        dj = kt - 4 * qc
                        qo = 128 * dj if dj >= 0 else 0
                        lo = li * 512
                        if dj >= 0:
                            # causal mask on the diagonal block: keep
                            # j >= p, zero below — affine predicate on the
                            # (otherwise idle) GPSIMD engine
                            nc.gpsimd.affine_select(
                                out=pt_t[:, lo + qo:lo + qo + 128],
                                in_=pt_t[:, lo + qo:lo + qo + 128],
                                compare_op=mybir.AluOpType.is_ge,
                                fill=0.0, base=0,
                                pattern=[[1, 128]],
                                channel_multiplier=-1)
                        tiles.append((kt, qo, lo, pt_t))
                thunks.append(th)
            return thunks

        def pv_thunks(qc, h, tiles):
            """PV accumulation pairs + final normalization for one head."""
            q0 = qc * 512
            hp = (h % 2) * 64
            nkt0 = 4 * qc + 4
            state = {}

            def pv_pair(pi):
                if "po" not in state:
                    state["po"] = pjo.tile([HD + 1, 512], F32, tag="pjo",
                                           name="pot")
                po_t = state["po"]
                for kt, qo, lo, pt_t in tiles[2 * pi:2 * pi + 2]:
                    nc.tensor.matmul(
                        po_t[:, qo:512],
                        Vp[:, kt * (HD + 1):(kt + 1) * (HD + 1)],
                        pt_t[:, lo + qo:lo + 512],
                        start=(kt == 0), stop=(kt == nkt0 - 1))
                if 2 * pi + 2 >= nkt0:
                    rc = small.tile([1, 512], F32, tag="rc", name="rc")
                    rb = small.tile([64, 512], F32, tag="rb", name="rb")
                    nc.vector.reciprocal(rc[:], po_t[64:65, :])
                    nc.gpsimd.partition_broadcast(rb[:], rc[:])
                    of = (h // 2) * S
                    nc.vector.tensor_mul(
                        OT[hp:hp + 64, of + q0:of + q0 + 512],
                        po_t[0:64, :], rb[:])

            return [lambda pi=pi: pv_pair(pi) for pi in range(nkt0 // 2)]

        def wo_half(qt, np2, half, obs, pool=None, ptag="pw",
                    act_copy=False, split_dma=False):
            """One 512-wide n-chunk; the second half fires the paired
            [128,1024] output DMA."""
            pool = pool or pw
            if half == 0:
                obs[(qt, np2)] = osb.tile([128, 1024], BF16, tag="ob",
                                          name="ob")
            ob = obs[(qt, np2)]
            ncn = 2 * np2 + half
            pw_t = pool.tile([128, 512], F32, tag=ptag, name="pw_t")
            for mt in range(2):
                nc.tensor.matmul(
                    pw_t[:],
                    OT[:, mt * S + qt * 128:mt * S + (qt + 1) * 128],
                    wo_sb[:, mt * DIM + ncn * 512:mt * DIM + ncn * 512 + 512],
                    start=(mt == 0), stop=(mt == 1))
            if act_copy:
                nc.scalar.copy(ob[:, half * 512:half * 512 + 512], pw_t[:])
            else:
                nc.vector.tensor_copy(
                    ob[:, half * 512:half * 512 + 512], pw_t[:])
            if split_dma:
                nc.sync.dma_start(
                    out[qt * 128:(qt + 1) * 128, ncn * 512:ncn * 512 + 512],
                    ob[:, half * 512:half * 512 + 512])
                if half == 1:
                    del obs[(qt, np2)]
                    wo_copy_flip[0] += 1
            elif half == 1:
                del obs[(qt, np2)]
                wo_copy_flip[0] += 1
                nc.sync.dma_start(
                    out[qt * 128:(qt + 1) * 128,
                        np2 * 1024:np2 * 1024 + 1024], ob[:])
        wo_obs = {}

        # ------------------------------------- merged emission schedule
        def merge(primary, *others):
            """Emit primary thunks; proportionally interleave the others."""
            counters = [0.0] * len(others)
            n = max(1, len(primary))
            for beat in primary:
                for j, lst in enumerate(others):
                    counters[j] += len(lst) / n
                    while counters[j] >= 1.0 and lst:
                        lst.pop(0)()
                        counters[j] -= 1.0
                for th in beat:
                    th()
            for lst in others:
                while lst:
                    lst.pop(0)()

        for th in proj_thunks(0, fused=True):       # prologue
            th()

        prev = None                      # (qc, h, tiles) awaiting PV
        for sc in range(NSC):
            if sc == 1:
                nc.sync.dma_start(wo_sb[:, 0:DIM], wo_s[0:128, :])
                nc.sync.dma_start(wo_sb[:, DIM:2 * DIM], wo_s[128:256, :])
            pstream = proj_thunks(sc + 1) if sc + 1 < NSC else []
            wostream = ([lambda qt=qt, np2=np2, half=half:
                         wo_half(qt, np2, half, wo_obs,
                                 act_copy=(sc == 3 and half == 1))
                         for qt in range(4 * (sc - 1), 4 * (sc - 1) + 4)
                         for np2 in range(2)
                         for half in range(2)] if sc >= 1 else [])
            for h in range(HQ):
                tiles = []
                sth = s_thunks(sc, h, tiles)
                pth = pv_thunks(*prev) if prev is not None else []
                beats = []
                for i in range(max(len(sth), len(pth))):
                    beat = []
                    if i < len(pth):
                        beat.append(pth[i])
                    if i < len(sth):
                        beat.append(sth[i])
                    beats.append(beat)
                if h == 0:
                    # front-load the next chunk's KV phase: the first score
                    # DRs wait on this chunk's Q8 (DVE rope chain), so give
                    # the PE independent work to chew first
                    ptake = min(len(pstream), 21)
                    wtake = (len(wostream) // HQ) if wostream else 0
                    merge(beats, pstream[:ptake], wostream[:wtake])
                    pstream = pstream[ptake:]
                    wostream = wostream[wtake:]
                else:
                    ptake = (len(pstream) // (HQ - h)) if pstream else 0
                    wtake = (len(wostream) // (HQ - h)) if wostream else 0
                    merge(beats, pstream[:ptake], wostream[:wtake])
                    pstream = pstream[ptake:]
                    wostream = wostream[wtake:]
                prev = (sc, h, tiles)
            merge([], pstream, wostream)

        # epilogue: PV of the last head, then WO of chunk 3
        for th in pv_thunks(*prev):
            th()
        epi = 0
        pools = [(pw, "pw"), (ps, "ps"), (pjo, "pjo")]
        for qt in range(12, 16):
            for np2 in range(2):
                pool, ptag = pools[epi % 3]
                for half in range(2):
                    wo_half(qt, np2, half, wo_obs, pool=pool, ptag=ptag,
                            act_copy=(half == 1), split_dma=True)
                epi += 1

    nc.compile()
    return nc


# ------------------------------------------------------------- host side
def _pair_perm64():
    """Column permutation putting the RoPE partner 16 partitions away."""
    return np.array([2 * (16 * (j // 32) + (j % 16)) + ((j % 32) // 16)
                     for j in range(64)])


def _f8(a):
    return np.ascontiguousarray(a.astype(ml_dtypes.float8_e4m3fn))


def _host_prep(x, freqs_cos, freqs_sin, wq, wk, wv, wo):
    x = np.asarray(x, np.float32)
    fc = np.asarray(freqs_cos, np.float32)
    fs = np.asarray(freqs_sin, np.float32)
    wq = np.asarray(wq, np.float32)
    wk = np.asarray(wk, np.float32)
    wv = np.asarray(wv, np.float32)
    wo = np.asarray(wo, np.float32)

    perm = _pair_perm64()
    xT = np.ascontiguousarray(x[0].T) * SX
    xT8 = _f8(xT)
    xTlo = _f8(xT - xT8.astype(np.float32))

    p = np.arange(64)
    pair = 16 * ((p % 64) // 32) + (p % 16)
    sign = np.where((p % 32) < 16, -1.0, 1.0).astype(np.float32)
    fold = SQ
    cosE = (np.ascontiguousarray(fc[:, pair].T) * fold).astype(
        ml_dtypes.bfloat16)                                      # [64, S]
    sinE = (np.ascontiguousarray(fs[:, pair].T) * sign[:, None] * fold
            ).astype(ml_dtypes.bfloat16)

    in_maps = []
    for c in range(NCORES):
        qcols = np.concatenate(
            [wq[:, (4 * c + i) * 64 + perm] for i in range(HQ)], axis=1)
        kcols = wk[:, c * 64 + perm]
        vcols = wv[:, c * 64:(c + 1) * 64]
        wqkv_c = np.concatenate([qcols, kcols, vcols], axis=1) * SW
        wqkv8 = _f8(wqkv_c)
        wqkvlo = _f8(wqkv_c - wqkv8.astype(np.float32))
        wo_c = wo[QW * c:QW * (c + 1), :].astype(ml_dtypes.bfloat16)
        in_maps.append({
            "xT8": xT8,
            "xTlo": xTlo,
            "wqkv8": wqkv8,
            "wqkvlo": wqkvlo,
            "wo_s": np.ascontiguousarray(wo_c),
            "cosE": cosE,
            "sinE": np.ascontiguousarray(sinE),
        })
    return in_maps


_NC_CACHE = {}


def get_program():
    if "v2" not in _NC_CACHE:
        _NC_CACHE["v2"] = build_program()
    return _NC_CACHE["v2"]


def kernel(x, freqs_cos, freqs_sin, wq, wk, wv, wo):
    nc = get_program()
    in_maps = _host_prep(x, freqs_cos, freqs_sin, wq, wk, wv, wo)
    res = run_bass_kernel_spmd(nc, in_maps, core_ids=list(range(NCORES)))
    acc = np.zeros((S, DIM), np.float32)
    for r in res.results:
        acc += np.asarray(r["out"], dtype=np.float32)
    return acc.reshape(1, S, DIM)


# revision 59
# speedup vs baseline: 1.0229x; 1.0229x over previous
"""Trainium2 Bass kernel for a GQA attention layer (dense transformer).

Reference computation (B=1, S=2048, DIM=2048, 32 q-heads, 8 kv-heads, hd=64):
    xq = x @ wq; xk = x @ wk; xv = x @ wv
    rope(xq, xk); GQA causal attention; out = attn @ wo

Sharding: tensor-parallel over heads across 8 cores. Core c owns q-heads
4c..4c+3 (wq cols), kv-head c (wk/wv cols), and wo rows 256c..256c+255.
Each core computes a full [S, DIM] partial of the output projection; the
host sums the 8 partials (the TP all-reduce, done at gather time).

fp8 strategy (e4m3 + DoubleRow = 0.5 PE-cycles/row over 2x128 contraction):
  - Projections: split-3 hi/lo fp8 (x = x8+xlo, w = w8+wlo, host-prepped,
    pow2-scaled).  x8@w8 + xlo@w8 per k-slice packed in one DoubleRow
    (x-side exact); x8@wlo over k-slice pairs.  0.75x of bf16 PE cost,
    error ~0.1%.
  - Scores: one DoubleRow per tile with slices (K8*Q8 + Klo*Q8): K-side
    bf16-exact (hi/lo planes), Q-side plain e4m3 with a stride-0 broadcast
    AP as the moving tensor (~1.6% rel err, inside the 2e-2 budget).
    0.5x of bf16 PE cost.
  - PV / WO stay bf16 (P residuals are unaffordable on-chip).

Kernel layout strategy (everything "transposed", head_dim on partitions):
  - RoPE pairs de-interleaved by permuting wq/wk columns on the host so the
    rotation partner sits 16 partitions away; pow2 scales folded into the
    cos/sin constants.
  - Scores computed transposed: S^T[k, q] per 128-row k-tile; exp on ACT
    (scale fused); causal mask = upper-tri multiply on diagonal blocks.
  - P@V as V'.T @ P^T with V' = [V | ones] (row 64 = softmax denominator).
  - Normalization: reciprocal (DVE) + partition_broadcast (GPSIMD) + mult.
  - RoPE inputs pulled PSUM->SBUF bf16 by ACT (activation Copy, unscaling
    fused) so the DVE rope ops run in 2-byte all-SBUF fast modes; causal
    masking runs on GPSIMD as an affine_select predicate.
  - Output projection from O^T with wo shard as rhs; partial DMA'd bf16
    (summed f32 on the host).
"""

import numpy as np
import ml_dtypes

import concourse.bass as bass
import concourse.mybir as mybir
from concourse import bacc
from concourse.tile import TileContext
from concourse.masks import make_identity
from concourse.bass_utils import run_bass_kernel_spmd

# ---------------------------------------------------------------- constants
S = 2048          # sequence length
DIM = 2048        # model dim
NH = 32           # query heads
NKV = 8           # kv heads
HD = 64           # head dim
NCORES = 8
HQ = NH // NCORES          # query heads per core = 4
QW = HQ * HD               # q width per core = 256
KT_S = S // 128            # 16 seq k-tiles
KT_D = DIM // 128          # 16 dim k-tiles
NSC = S // 512             # 4 s-chunks
SCALE = 1.0 / 8.0          # 1/sqrt(64)

SX = 16.0                  # x fp8 scale
SW = 1024.0                # wq/wk/wv fp8 scale
SQ = 16.0                  # q/k fp8 scale (shared so one cos/sin set works)

WQKV = QW + 2 * HD  # 384

_SHUF_SWAP16 = list(range(16, 32)) + list(range(16))

F8 = mybir.dt.float8e4
BF16 = mybir.dt.bfloat16
F32 = mybir.dt.float32
DR = mybir.MatmulPerfMode.DoubleRow
EXP = mybir.ActivationFunctionType.Exp
MULT = mybir.AluOpType.mult
ADD = mybir.AluOpType.add


def build_program():
    """Build the per-core Bass program (same program on all 8 cores).

    Emission is a fine-grained software pipeline: attention beats for chunk
    sc (S^T matmuls for head h + PV pairs of head h-1) are merged with the
    projection matmuls of chunk sc+1 and the WO units of chunk sc-1.

    PSUM (8 banks): pjo 3 (projection passes + PV accumulators, shared tag)
    + ps 4 (two [128,1024] score megas) + pw 1 (WO + V/warm transposes).
    """
    nc = bacc.Bacc("TRN2", target_bir_lowering=False, debug=False,
                   num_devices=NCORES)

    xT8 = nc.dram_tensor("xT8", [DIM, S], F8, kind="ExternalInput")
    xTlo = nc.dram_tensor("xTlo", [DIM, S], F8, kind="ExternalInput")
    wqkv8 = nc.dram_tensor("wqkv8", [DIM, WQKV], F8, kind="ExternalInput")
    wqkvlo = nc.dram_tensor("wqkvlo", [DIM, WQKV], F8, kind="ExternalInput")
    wo_s = nc.dram_tensor("wo_s", [QW, DIM], BF16, kind="ExternalInput")
    cosE = nc.dram_tensor("cosE", [64, S], BF16, kind="ExternalInput")
    sinE = nc.dram_tensor("sinE", [64, S], BF16, kind="ExternalInput")
    out = nc.dram_tensor("out", [S, DIM], BF16, kind="ExternalOutput")

    import contextlib
    with TileContext(nc) as tc, contextlib.ExitStack() as ctx:
        const = ctx.enter_context(tc.tile_pool(name="const", bufs=1))
        work = ctx.enter_context(tc.tile_pool(name="work", bufs=3))
        xtp = ctx.enter_context(tc.tile_pool(name="xtp", bufs=8))
        ptp = ctx.enter_context(tc.tile_pool(name="ptp", bufs=21))
        small = ctx.enter_context(tc.tile_pool(name="small", bufs=7))
        osb = ctx.enter_context(tc.tile_pool(name="osb", bufs=6))

        pjo = ctx.enter_context(tc.tile_pool(name="pjo", bufs=3,
                                             space="PSUM"))
        ps = ctx.enter_context(tc.tile_pool(name="ps", bufs=2, space="PSUM"))
        pw = ctx.enter_context(tc.tile_pool(name="pw", bufs=1, space="PSUM"))

        # ----------------------------------------------- persistent SBUF
        w8 = const.tile([128, KT_D, WQKV], F8, tag="w8")
        wlo = const.tile([128, KT_D, WQKV], F8, tag="wlo")
        wo_sb = const.tile([128, 2 * DIM], BF16, tag="wo_sb")
        cos_sb = const.tile([128, S], BF16, tag="cos_sb")
        sin_sb = const.tile([128, S], BF16, tag="sin_sb")
        ident = const.tile([128, 128], BF16, tag="ident")
        Q8 = const.tile([64, HQ * S], F8, tag="Q8")
        # K planes: [:,0,:] = K8 (hi), [:,1,:] = Klo (residual): the
        # DoubleRow lhsT slices make the K side bf16-exact; Q is plain e4m3
        K8 = const.tile([64, 2, S], F8, tag="K8")
        Vt = const.tile([128, S], BF16, tag="Vt")  # V in rows 64:128
        Vp = const.tile([128, KT_S * (HD + 1)], BF16, tag="Vp")
        OT = const.tile([128, 2 * S], BF16, tag="OT")

        make_identity(nc, ident[:])
        nc.gpsimd.memset(Vp[:], 1.0)  # ones columns for denominator

        # PE warm-up: burn the pstate ramp on dependency-free transposes
        # while the first weight/x DMAs are in flight (each real matmul in
        # the first 3us of a busy window runs at half speed otherwise)
        warm = pw.tile([128, 128], BF16, tag="pw", name="warm")
        for _ in range(24):
            nc.tensor.transpose(warm[:], ident[:], ident[:])

        wo_copy_flip = [0]

        # ---------------------------------------------- thunk generators
        def proj_thunks(sc, fused=False):
            """Projection of chunk sc.

            Three DoubleRow passes per output slab, each packing k-slice
            pairs in the two DR slices: x8@w8, xlo@w8, x8@wlo.  All
            accumulate one PSUM tile per slab (KV / Q0 / Q1)."""
            s0 = sc * 512
            xts = []
            st = {}

            batches = [4, 4, 4, 4]
            starts = [sum(batches[:i]) for i in range(len(batches))]
            kt_slot = {}
            for bi, (b0, bn) in enumerate(zip(starts, batches)):
                for j in range(bn):
                    kt_slot[b0 + j] = (bi, j, bn, b0)

            def slab_off(slab):
                # slab 0,1 = Q cols [0:128],[128:256]; slab 2 = KV [256:384]
                return 256 if slab == 2 else slab * 128

            def drp(slab, pi, term):
                # k-slice pair (2pi, 2pi+1); term 0: x8@w8, 1: xlo@w8,
                # 2: x8@wlo
                kt0 = 2 * pi
                bi, j, bn, b0 = kt_slot[kt0]
                assert kt_slot[kt0 + 1][0] == bi  # same batch tile
                xt = xts[bi]
                off = slab_off(slab)
                wt = wlo if term == 2 else w8
                xplane = 1 if term == 1 else 0
                nc.tensor.matmul(
                    st[f"p{slab}"][:],
                    wt[:, kt0:kt0 + 2, off:off + 128],
                    xt[:, xplane, j * 512:j * 512 + 1024].rearrange(
                        "p (k f) -> p k f", k=2),
                    start=(pi == 0 and term == 0),
                    stop=(pi == KT_D // 2 - 1 and term == 2),
                    perf_mode=DR)

            def dma_kv(kt):
                bi, j, bn, b0 = kt_slot[kt]
                if j == 0:
                    if sc == 0:
                        nc.sync.dma_start(
                            w8[:, b0:b0 + bn, :],
                            wqkv8[b0 * 128:(b0 + bn) * 128, :].rearrange(
                                "(k r) w -> r k w", k=bn))
                        nc.sync.dma_start(
                            wlo[:, b0:b0 + bn, :],
                            wqkvlo[b0 * 128:(b0 + bn) * 128, :].rearrange(
                                "(k r) w -> r k w", k=bn))
                    xt4 = xtp.tile([128, 2, 4 * 512], F8, tag="xt",
                                   name="xt4")
                    nc.sync.dma_start(
                        xt4[:, 0, 0:bn * 512].rearrange(
                            "r (k c) -> r k c", k=bn),
                        xT8[b0 * 128:(b0 + bn) * 128,
                            s0:s0 + 512].rearrange("(k r) c -> r k c", k=bn))
                    nc.sync.dma_start(
                        xt4[:, 1, 0:bn * 512].rearrange(
                            "r (k c) -> r k c", k=bn),
                        xTlo[b0 * 128:(b0 + bn) * 128,
                             s0:s0 + 512].rearrange("(k r) c -> r k c", k=bn))
                    xts.append(xt4)
                    if kt == (4 if sc == 0 else 0):
                        # stream only this chunk's cos/sin columns
                        for tsb, tdr in ((cos_sb, cosE), (sin_sb, sinE)):
                            nc.sync.dma_start(
                                tsb[0:64, s0:s0 + 512],
                                tdr[:, s0:s0 + 512])
                            nc.sync.dma_start(
                                tsb[64:128, s0:s0 + 512],
                                tdr[:, s0:s0 + 512])
                if kt == 0:
                    st["p2"] = pjo.tile([128, 512], F32, tag="pjo",
                                        name="pkv")
                    if fused:
                        st["p0"] = pjo.tile([128, 512], F32, tag="pjo",
                                            name="fq0")
                        st["p1"] = pjo.tile([128, 512], F32, tag="pjo",
                                            name="fq1")
                if kt % 2 == 1:
                    for term in range(3):
                        drp(2, kt // 2, term)
                        if fused:
                            for mt in range(2):
                                drp(mt, kt // 2, term)
                if fused and kt % 4 == 3 and kt < 15:
                    # prologue is DMA-paced: each batch's DRs underfill the
                    # PE vs its ~3us DMA; top up with free transposes so the
                    # engine neither idles nor cools down
                    for _ in range(10):
                        nc.tensor.transpose(warm[:], ident[:], ident[:])

            def k_rope():
                pkv = st["p2"]
                # ACT pulls the PSUM slab to SBUF bf16 (unscaling sx*sw) so
                # every DVE op below runs in 2-byte all-SBUF 4x mode
                ks = work.tile([128, 512], BF16, tag="ks", name="ks")
                nc.scalar.activation(ks[:], pkv[:],
                                     mybir.ActivationFunctionType.Copy,
                                     scale=1.0 / (SX * SW))
                shufk = work.tile([64, 512], BF16, tag="shufk", name="shufk")
                m1k = work.tile([64, 512], BF16, tag="m1k", name="m1k")
                t2k = work.tile([64, 512], BF16, tag="t2k", name="t2k")
                kb = work.tile([64, 512], BF16, tag="kb", name="kb")
                nc.vector.stream_shuffle(shufk[:], ks[0:64, :],
                                         _SHUF_SWAP16)
                nc.vector.tensor_mul(m1k[:], ks[0:64, :],
                                     cos_sb[0:64, s0:s0 + 512])
                nc.vector.tensor_mul(t2k[:], shufk[:],
                                     sin_sb[0:64, s0:s0 + 512])
                nc.vector.tensor_add(kb[:], m1k[:], t2k[:])
                nc.vector.tensor_copy(K8[:, 0, s0:s0 + 512], kb[:])
                nc.vector.scalar_tensor_tensor(
                    K8[:, 1, s0:s0 + 512], K8[:, 0, s0:s0 + 512], -1.0,
                    kb[:], op0=MULT, op1=ADD)
                nc.vector.tensor_copy(Vt[64:128, s0:s0 + 512],
                                      ks[64:128, :])

            def v_trans(kt):
                pv = pw.tile([128, HD], BF16, tag="pw", name="pv")
                nc.tensor.transpose(
                    pv[:], Vt[64:128, kt * 128:(kt + 1) * 128],
                    ident[64:128, 64:128])
                nc.vector.tensor_copy(
                    Vp[:, kt * (HD + 1):kt * (HD + 1) + HD], pv[:])

            def q_mm(mt, kt):
                if kt == 0:
                    st[f"p{mt}"] = pjo.tile([128, 512], F32, tag="pjo",
                                            name="pq")
                if kt % 2 == 1:
                    for term in range(3):
                        drp(mt, kt // 2, term)

            def q_rope(mt):
                pq = st[f"p{mt}"]
                qs = work.tile([128, 512], BF16, tag="qs", name="qs")
                nc.scalar.activation(qs[:], pq[:],
                                     mybir.ActivationFunctionType.Copy,
                                     scale=1.0 / (SX * SW))
                shuf = work.tile([128, 512], BF16, tag="shuf", name="shuf")
                m1 = work.tile([128, 512], BF16, tag="m1", name="m1")
                t2 = work.tile([128, 512], BF16, tag="t2", name="t2")
                nc.vector.stream_shuffle(shuf[:], qs[:], _SHUF_SWAP16)
                nc.vector.tensor_mul(m1[:], qs[:], cos_sb[:, s0:s0 + 512])
                nc.vector.tensor_mul(t2[:], shuf[:], sin_sb[:, s0:s0 + 512])
                he = (2 * mt) * S
                ho = (2 * mt + 1) * S
                # fp8 write directly from the rope add (plain-e4m3 Q side)
                nc.vector.tensor_add(Q8[:, he + s0:he + s0 + 512],
                                     m1[0:64, :], t2[0:64, :])
                nc.vector.tensor_add(Q8[:, ho + s0:ho + s0 + 512],
                                     m1[64:128, :], t2[64:128, :])

            th = [lambda kt=kt: dma_kv(kt) for kt in range(KT_D)]
            if fused:
                th += [lambda: q_rope(0), k_rope, lambda: q_rope(1)]
                th += [lambda kt=kt: v_trans(kt)
                       for kt in range(4 * sc, 4 * sc + 4)]
                return th
            th.append(k_rope)
            th += [lambda kt=kt: v_trans(kt)
                   for kt in range(4 * sc, 4 * sc + 4)]
            if True:
                for mt in range(2):
                    th += [lambda mt=mt, kt=kt: q_mm(mt, kt)
                           for kt in range(KT_D)]
                    th.append(lambda mt=mt: q_rope(mt))
            return th

        def s_thunks(qc, h, tiles):
            """S^T DoubleRow matmuls + exp + mask for one head."""
            q0 = qc * 512
            hf = h * S
            nkt = 4 * qc + 4
            thunks = []
            for pi in range(nkt // 2):
                def th(pi=pi):
                    kts = (2 * pi, 2 * pi + 1)
                    ps_t = ps.tile([128, 1024], F32, tag="ps", name="ps_t")
                    pt_t = ptp.tile([128, 1024], BF16, tag="pt", name="pt_t")
                    for li, kt in enumerate(kts):
                        dj = kt - 4 * qc
                        qo = 128 * dj if dj >= 0 else 0
                        if 2 * pi == 4 * qc:
                            qo = 0  # keep the merged full-width exp legal
                        lo = li * 512
                        nc.tensor.matmul(
                            ps_t[:, lo + qo:lo + 512],
                            K8[:, :, kt * 128:(kt + 1) * 128],
                            Q8[:, hf + q0 + qo:hf + q0 + 512]
                            .unsqueeze(1).broadcast_to((64, 2, 512 - qo)),
                            start=True, stop=True, perf_mode=DR)
                    if 2 * pi + 1 < 4 * qc or 2 * pi == 4 * qc:
                        # interior pair, or the (dj0,dj1) boundary pair:
                        # one full-width exp (the dj1 junk region is never
                        # read downstream) — fewer ACT instructions
                        nc.scalar.activation(
                            pt_t[:], ps_t[:], EXP,
                            scale=SCALE / (SQ * SQ))
                    else:
                        for li, kt in enumerate(kts):
                            dj = kt - 4 * qc
                            qo = 128 * dj if dj >= 0 else 0
                            lo = li * 512
                            nc.scalar.activation(
                                pt_t[:, lo + qo:lo + 512],
                                ps_t[:, lo + qo:lo + 512], EXP,
                                scale=SCALE / (SQ * SQ))
                    for li, kt in enumerate(kts):
                        dj = kt - 4 * qc
                        qo = 128 * dj if dj >= 0 else 0
                        lo = li * 512
                        if dj >= 0:
                            # causal mask on the diagonal block: keep
                            # j >= p, zero below — affine predicate on the
                            # (otherwise idle) GPSIMD engine
                            nc.gpsimd.affine_select(
                                out=pt_t[:, lo + qo:lo + qo + 128],
                                in_=pt_t[:, lo + qo:lo + qo + 128],
                                compare_op=mybir.AluOpType.is_ge,
                                fill=0.0, base=0,
                                pattern=[[1, 128]],
                                channel_multiplier=-1)
                        tiles.append((kt, qo, lo, pt_t))
                thunks.append(th)
            return thunks

        def pv_thunks(qc, h, tiles):
            """PV accumulation pairs + final normalization for one head."""
            q0 = qc * 512
            hp = (h % 2) * 64
            nkt0 = 4 * qc + 4
            state = {}

            def pv_pair(pi):
                if "po" not in state:
                    state["po"] = pjo.tile([HD + 1, 512], F32, tag="pjo",
                                           name="pot")
                po_t = state["po"]
                for kt, qo, lo, pt_t in tiles[2 * pi:2 * pi + 2]:
                    nc.tensor.matmul(
                        po_t[:, qo:512],
                        Vp[:, kt * (HD + 1):(kt + 1) * (HD + 1)],
                        pt_t[:, lo + qo:lo + 512],
                        start=(kt == 0), stop=(kt == nkt0 - 1))
                if 2 * pi + 2 >= nkt0:
                    rc = small.tile([1, 512], F32, tag="rc", name="rc")
                    rb = small.tile([64, 512], F32, tag="rb", name="rb")
                    nc.vector.reciprocal(rc[:], po_t[64:65, :])
                    nc.gpsimd.partition_broadcast(rb[:], rc[:])
                    of = (h // 2) * S
                    nc.vector.tensor_mul(
                        OT[hp:hp + 64, of + q0:of + q0 + 512],
                        po_t[0:64, :], rb[:])

            return [lambda pi=pi: pv_pair(pi) for pi in range(nkt0 // 2)]

        def wo_half(qt, np2, half, obs, pool=None, ptag="pw",
                    act_copy=False, split_dma=False):
            """One 512-wide n-chunk; the second half fires the paired
            [128,1024] output DMA."""
            pool = pool or pw
            if half == 0:
                obs[(qt, np2)] = osb.tile([128, 1024], BF16, tag="ob",
                                          name="ob")
            ob = obs[(qt, np2)]
            ncn = 2 * np2 + half
            pw_t = pool.tile([128, 512], F32, tag=ptag, name="pw_t")
            for mt in range(2):
                nc.tensor.matmul(
                    pw_t[:],
                    OT[:, mt * S + qt * 128:mt * S + (qt + 1) * 128],
                    wo_sb[:, mt * DIM + ncn * 512:mt * DIM + ncn * 512 + 512],
                    start=(mt == 0), stop=(mt == 1))
            if act_copy:
                nc.scalar.copy(ob[:, half * 512:half * 512 + 512], pw_t[:])
            else:
                nc.vector.tensor_copy(
                    ob[:, half * 512:half * 512 + 512], pw_t[:])
            if split_dma:
                nc.sync.dma_start(
                    out[qt * 128:(qt + 1) * 128, ncn * 512:ncn * 512 + 512],
                    ob[:, half * 512:half * 512 + 512])
                if half == 1:
                    del obs[(qt, np2)]
                    wo_copy_flip[0] += 1
            elif half == 1:
                del obs[(qt, np2)]
                wo_copy_flip[0] += 1
                nc.sync.dma_start(
                    out[qt * 128:(qt + 1) * 128,
                        np2 * 1024:np2 * 1024 + 1024], ob[:])
        wo_obs = {}

        # ------------------------------------- merged emission schedule
        def merge(primary, *others):
            """Emit primary thunks; proportionally interleave the others."""
            counters = [0.0] * len(others)
            n = max(1, len(primary))
            for beat in primary:
                for j, lst in enumerate(others):
                    counters[j] += len(lst) / n
                    while counters[j] >= 1.0 and lst:
                        lst.pop(0)()
                        counters[j] -= 1.0
                for th in beat:
                    th()
            for lst in others:
                while lst:
                    lst.pop(0)()

        for th in proj_thunks(0, fused=True):       # prologue
            th()

        prev = None                      # (qc, h, tiles) awaiting PV
        for sc in range(NSC):
            if sc == 1:
                nc.sync.dma_start(wo_sb[:, 0:DIM], wo_s[0:128, :])
                nc.sync.dma_start(wo_sb[:, DIM:2 * DIM], wo_s[128:256, :])
            pstream = proj_thunks(sc + 1) if sc + 1 < NSC else []
            wostream = ([lambda qt=qt, np2=np2, half=half:
                         wo_half(qt, np2, half, wo_obs)
                         for qt in range(4 * (sc - 1), 4 * (sc - 1) + 4)
                         for np2 in range(2)
                         for half in range(2)] if sc >= 1 else [])
            for h in range(HQ):
                tiles = []
                sth = s_thunks(sc, h, tiles)
                pth = pv_thunks(*prev) if prev is not None else []
                beats = []
                for i in range(max(len(sth), len(pth))):
                    beat = []
                    if i < len(pth):
                        beat.append(pth[i])
                    if i < len(sth):
                        beat.append(sth[i])
                    beats.append(beat)
                if h == 0:
                    # front-load the next chunk's KV phase: the first score
                    # DRs wait on this chunk's Q8 (DVE rope chain), so give
                    # the PE independent work to chew first
                    ptake = min(len(pstream), 21)
                    wtake = (len(wostream) // HQ) if wostream else 0
                    merge(beats, pstream[:ptake], wostream[:wtake])
                    pstream = pstream[ptake:]
                    wostream = wostream[wtake:]
                else:
                    ptake = (len(pstream) // (HQ - h)) if pstream else 0
                    wtake = (len(wostream) // (HQ - h)) if wostream else 0
                    merge(beats, pstream[:ptake], wostream[:wtake])
                    pstream = pstream[ptake:]
                    wostream = wostream[wtake:]
                prev = (sc, h, tiles)
            merge([], pstream, wostream)

        # epilogue: PV of the last head, then WO of chunk 3
        for th in pv_thunks(*prev):
            th()
        epi = 0
        pools = [(pw, "pw"), (ps, "ps"), (pjo, "pjo")]
        for qt in range(12, 16):
            for np2 in range(2):
                pool, ptag = pools[epi % 3]
                for half in range(2):
                    wo_half(qt, np2, half, wo_obs, pool=pool, ptag=ptag,
                            act_copy=(half == 1), split_dma=True)
                epi += 1

    nc.compile()
    return nc


# ------------------------------------------------------------- host side
def _pair_perm64():
    """Column permutation putting the RoPE partner 16 partitions away."""
    return np.array([2 * (16 * (j // 32) + (j % 16)) + ((j % 32) // 16)
                     for j in range(64)])


def _f8(a):
    return np.ascontiguousarray(a.astype(ml_dtypes.float8_e4m3fn))


def _host_prep(x, freqs_cos, freqs_sin, wq, wk, wv, wo):
    x = np.asarray(x, np.float32)
    fc = np.asarray(freqs_cos, np.float32)
    fs = np.asarray(freqs_sin, np.float32)
    wq = np.asarray(wq, np.float32)
    wk = np.asarray(wk, np.float32)
    wv = np.asarray(wv, np.float32)
    wo = np.asarray(wo, np.float32)

    perm = _pair_perm64()
    xT = np.ascontiguousarray(x[0].T) * SX
    xT8 = _f8(xT)
    xTlo = _f8(xT - xT8.astype(np.float32))

    p = np.arange(64)
    pair = 16 * ((p % 64) // 32) + (p % 16)
    sign = np.where((p % 32) < 16, -1.0, 1.0).astype(np.float32)
    fold = SQ
    cosE = (np.ascontiguousarray(fc[:, pair].T) * fold).astype(
        ml_dtypes.bfloat16)                                      # [64, S]
    sinE = (np.ascontiguousarray(fs[:, pair].T) * sign[:, None] * fold
            ).astype(ml_dtypes.bfloat16)

    in_maps = []
    for c in range(NCORES):
        qcols = np.concatenate(
            [wq[:, (4 * c + i) * 64 + perm] for i in range(HQ)], axis=1)
        kcols = wk[:, c * 64 + perm]
        vcols = wv[:, c * 64:(c + 1) * 64]
        wqkv_c = np.concatenate([qcols, kcols, vcols], axis=1) * SW
        wqkv8 = _f8(wqkv_c)
        wqkvlo = _f8(wqkv_c - wqkv8.astype(np.float32))
        wo_c = wo[QW * c:QW * (c + 1), :].astype(ml_dtypes.bfloat16)
        in_maps.append({
            "xT8": xT8,
            "xTlo": xTlo,
            "wqkv8": wqkv8,
            "wqkvlo": wqkvlo,
            "wo_s": np.ascontiguousarray(wo_c),
            "cosE": cosE,
            "sinE": np.ascontiguousarray(sinE),
        })
    return in_maps


_NC_CACHE = {}


def get_program():
    if "v2" not in _NC_CACHE:
        _NC_CACHE["v2"] = build_program()
    return _NC_CACHE["v2"]


def kernel(x, freqs_cos, freqs_sin, wq, wk, wv, wo):
    nc = get_program()
    in_maps = _host_prep(x, freqs_cos, freqs_sin, wq, wk, wv, wo)
    res = run_bass_kernel_spmd(nc, in_maps, core_ids=list(range(NCORES)))
    acc = np.zeros((S, DIM), np.float32)
    for r in res.results:
        acc += np.asarray(r["out"], dtype=np.float32)
    return acc.reshape(1, S, DIM)


# revision 60
# speedup vs baseline: 1.0255x; 1.0026x over previous
"""Trainium2 Bass kernel for a GQA attention layer (dense transformer).

Reference computation (B=1, S=2048, DIM=2048, 32 q-heads, 8 kv-heads, hd=64):
    xq = x @ wq; xk = x @ wk; xv = x @ wv
    rope(xq, xk); GQA causal attention; out = attn @ wo

Sharding: tensor-parallel over heads across 8 cores. Core c owns q-heads
4c..4c+3 (wq cols), kv-head c (wk/wv cols), and wo rows 256c..256c+255.
Each core computes a full [S, DIM] partial of the output projection; the
host sums the 8 partials (the TP all-reduce, done at gather time).

fp8 strategy (e4m3 + DoubleRow = 0.5 PE-cycles/row over 2x128 contraction):
  - Projections: split-3 hi/lo fp8 (x = x8+xlo, w = w8+wlo, host-prepped,
    pow2-scaled).  x8@w8 + xlo@w8 per k-slice packed in one DoubleRow
    (x-side exact); x8@wlo over k-slice pairs.  0.75x of bf16 PE cost,
    error ~0.1%.
  - Scores: one DoubleRow per tile with slices (K8*Q8 + Klo*Q8): K-side
    bf16-exact (hi/lo planes), Q-side plain e4m3 with a stride-0 broadcast
    AP as the moving tensor (~1.6% rel err, inside the 2e-2 budget).
    0.5x of bf16 PE cost.
  - PV / WO stay bf16 (P residuals are unaffordable on-chip).

Kernel layout strategy (everything "transposed", head_dim on partitions):
  - RoPE pairs de-interleaved by permuting wq/wk columns on the host so the
    rotation partner sits 16 partitions away; pow2 scales folded into the
    cos/sin constants.
  - Scores computed transposed: S^T[k, q] per 128-row k-tile; exp on ACT
    (scale fused); causal mask = upper-tri multiply on diagonal blocks.
  - P@V as V'.T @ P^T with V' = [V | ones] (row 64 = softmax denominator).
  - Normalization: reciprocal (DVE) + partition_broadcast (GPSIMD) + mult.
  - RoPE inputs pulled PSUM->SBUF bf16 by ACT (activation Copy, unscaling
    fused) so the DVE rope ops run in 2-byte all-SBUF fast modes; causal
    masking runs on GPSIMD as an affine_select predicate.
  - Output projection from O^T with wo shard as rhs; partial DMA'd bf16
    (summed f32 on the host).
"""

import numpy as np
import ml_dtypes

import concourse.bass as bass
import concourse.mybir as mybir
from concourse import bacc
from concourse.tile import TileContext
from concourse.masks import make_identity
from concourse.bass_utils import run_bass_kernel_spmd

# ---------------------------------------------------------------- constants
S = 2048          # sequence length
DIM = 2048        # model dim
NH = 32           # query heads
NKV = 8           # kv heads
HD = 64           # head dim
NCORES = 8
HQ = NH // NCORES          # query heads per core = 4
QW = HQ * HD               # q width per core = 256
KT_S = S // 128            # 16 seq k-tiles
KT_D = DIM // 128          # 16 dim k-tiles
NSC = S // 512             # 4 s-chunks
SCALE = 1.0 / 8.0          # 1/sqrt(64)

SX = 16.0                  # x fp8 scale
SW = 1024.0                # wq/wk/wv fp8 scale
SQ = 16.0                  # q/k fp8 scale (shared so one cos/sin set works)

WQKV = QW + 2 * HD  # 384

_SHUF_SWAP16 = list(range(16, 32)) + list(range(16))

F8 = mybir.dt.float8e4
BF16 = mybir.dt.bfloat16
F32 = mybir.dt.float32
DR = mybir.MatmulPerfMode.DoubleRow
EXP = mybir.ActivationFunctionType.Exp
MULT = mybir.AluOpType.mult
ADD = mybir.AluOpType.add


def build_program():
    """Build the per-core Bass program (same program on all 8 cores).

    Emission is a fine-grained software pipeline: attention beats for chunk
    sc (S^T matmuls for head h + PV pairs of head h-1) are merged with the
    projection matmuls of chunk sc+1 and the WO units of chunk sc-1.

    PSUM (8 banks): pjo 3 (projection passes + PV accumulators, shared tag)
    + ps 4 (two [128,1024] score megas) + pw 1 (WO + V/warm transposes).
    """
    nc = bacc.Bacc("TRN2", target_bir_lowering=False, debug=False,
                   num_devices=NCORES)

    xT8 = nc.dram_tensor("xT8", [DIM, S], F8, kind="ExternalInput")
    xTlo = nc.dram_tensor("xTlo", [DIM, S], F8, kind="ExternalInput")
    wqkv8 = nc.dram_tensor("wqkv8", [DIM, WQKV], F8, kind="ExternalInput")
    wqkvlo = nc.dram_tensor("wqkvlo", [DIM, WQKV], F8, kind="ExternalInput")
    wo_s = nc.dram_tensor("wo_s", [QW, DIM], BF16, kind="ExternalInput")
    cosE = nc.dram_tensor("cosE", [64, S], BF16, kind="ExternalInput")
    sinE = nc.dram_tensor("sinE", [64, S], BF16, kind="ExternalInput")
    out = nc.dram_tensor("out", [S, DIM], BF16, kind="ExternalOutput")

    import contextlib
    with TileContext(nc) as tc, contextlib.ExitStack() as ctx:
        const = ctx.enter_context(tc.tile_pool(name="const", bufs=1))
        work = ctx.enter_context(tc.tile_pool(name="work", bufs=3))
        xtp = ctx.enter_context(tc.tile_pool(name="xtp", bufs=8))
        ptp = ctx.enter_context(tc.tile_pool(name="ptp", bufs=21))
        small = ctx.enter_context(tc.tile_pool(name="small", bufs=7))
        osb = ctx.enter_context(tc.tile_pool(name="osb", bufs=6))

        pjo = ctx.enter_context(tc.tile_pool(name="pjo", bufs=3,
                                             space="PSUM"))
        ps = ctx.enter_context(tc.tile_pool(name="ps", bufs=2, space="PSUM"))
        pw = ctx.enter_context(tc.tile_pool(name="pw", bufs=1, space="PSUM"))

        # ----------------------------------------------- persistent SBUF
        w8 = const.tile([128, KT_D, WQKV], F8, tag="w8")
        wlo = const.tile([128, KT_D, WQKV], F8, tag="wlo")
        wo_sb = const.tile([128, 2 * DIM], BF16, tag="wo_sb")
        cos_sb = const.tile([128, S], BF16, tag="cos_sb")
        sin_sb = const.tile([128, S], BF16, tag="sin_sb")
        ident = const.tile([128, 128], BF16, tag="ident")
        Q8 = const.tile([64, HQ * S], F8, tag="Q8")
        # K planes: [:,0,:] = K8 (hi), [:,1,:] = Klo (residual): the
        # DoubleRow lhsT slices make the K side bf16-exact; Q is plain e4m3
        K8 = const.tile([64, 2, S], F8, tag="K8")
        Vt = const.tile([128, S], BF16, tag="Vt")  # V in rows 64:128
        Vp = const.tile([128, KT_S * (HD + 1)], BF16, tag="Vp")
        OT = const.tile([128, 2 * S], BF16, tag="OT")

        make_identity(nc, ident[:])
        nc.gpsimd.memset(Vp[:], 1.0)  # ones columns for denominator

        # PE warm-up: burn the pstate ramp on dependency-free transposes
        # while the first weight/x DMAs are in flight (each real matmul in
        # the first 3us of a busy window runs at half speed otherwise)
        warm = pw.tile([128, 128], BF16, tag="pw", name="warm")
        for _ in range(24):
            nc.tensor.transpose(warm[:], ident[:], ident[:])

        wo_copy_flip = [0]

        # ---------------------------------------------- thunk generators
        def proj_thunks(sc, fused=False):
            """Projection of chunk sc.

            Three DoubleRow passes per output slab, each packing k-slice
            pairs in the two DR slices: x8@w8, xlo@w8, x8@wlo.  All
            accumulate one PSUM tile per slab (KV / Q0 / Q1)."""
            s0 = sc * 512
            xts = []
            st = {}

            batches = [4, 4, 4, 4]
            starts = [sum(batches[:i]) for i in range(len(batches))]
            kt_slot = {}
            for bi, (b0, bn) in enumerate(zip(starts, batches)):
                for j in range(bn):
                    kt_slot[b0 + j] = (bi, j, bn, b0)

            def slab_off(slab):
                # slab 0,1 = Q cols [0:128],[128:256]; slab 2 = KV [256:384]
                return 256 if slab == 2 else slab * 128

            def drp(slab, pi, term):
                # k-slice pair (2pi, 2pi+1); term 0: x8@w8, 1: xlo@w8,
                # 2: x8@wlo
                kt0 = 2 * pi
                bi, j, bn, b0 = kt_slot[kt0]
                assert kt_slot[kt0 + 1][0] == bi  # same batch tile
                xt = xts[bi]
                off = slab_off(slab)
                wt = wlo if term == 2 else w8
                xplane = 1 if term == 1 else 0
                nc.tensor.matmul(
                    st[f"p{slab}"][:],
                    wt[:, kt0:kt0 + 2, off:off + 128],
                    xt[:, xplane, j * 512:j * 512 + 1024].rearrange(
                        "p (k f) -> p k f", k=2),
                    start=(pi == 0 and term == 0),
                    stop=(pi == KT_D // 2 - 1 and term == 2),
                    perf_mode=DR)

            def dma_kv(kt):
                bi, j, bn, b0 = kt_slot[kt]
                if j == 0:
                    if sc == 0:
                        nc.sync.dma_start(
                            w8[:, b0:b0 + bn, :],
                            wqkv8[b0 * 128:(b0 + bn) * 128, :].rearrange(
                                "(k r) w -> r k w", k=bn))
                        nc.sync.dma_start(
                            wlo[:, b0:b0 + bn, :],
                            wqkvlo[b0 * 128:(b0 + bn) * 128, :].rearrange(
                                "(k r) w -> r k w", k=bn))
                    xt4 = xtp.tile([128, 2, 4 * 512], F8, tag="xt",
                                   name="xt4")
                    nc.sync.dma_start(
                        xt4[:, 0, 0:bn * 512].rearrange(
                            "r (k c) -> r k c", k=bn),
                        xT8[b0 * 128:(b0 + bn) * 128,
                            s0:s0 + 512].rearrange("(k r) c -> r k c", k=bn))
                    nc.sync.dma_start(
                        xt4[:, 1, 0:bn * 512].rearrange(
                            "r (k c) -> r k c", k=bn),
                        xTlo[b0 * 128:(b0 + bn) * 128,
                             s0:s0 + 512].rearrange("(k r) c -> r k c", k=bn))
                    xts.append(xt4)
                    if kt == (4 if sc == 0 else 0):
                        # stream only this chunk's cos/sin columns
                        for tsb, tdr in ((cos_sb, cosE), (sin_sb, sinE)):
                            nc.sync.dma_start(
                                tsb[0:64, s0:s0 + 512],
                                tdr[:, s0:s0 + 512])
                            nc.sync.dma_start(
                                tsb[64:128, s0:s0 + 512],
                                tdr[:, s0:s0 + 512])
                if kt == 0:
                    st["p2"] = pjo.tile([128, 512], F32, tag="pjo",
                                        name="pkv")
                    if fused:
                        st["p0"] = pjo.tile([128, 512], F32, tag="pjo",
                                            name="fq0")
                        st["p1"] = pjo.tile([128, 512], F32, tag="pjo",
                                            name="fq1")
                if kt % 2 == 1:
                    for term in range(3):
                        drp(2, kt // 2, term)
                        if fused:
                            for mt in range(2):
                                drp(mt, kt // 2, term)
                if fused and kt % 4 == 3 and kt < 15:
                    # prologue is DMA-paced: each batch's DRs underfill the
                    # PE vs its ~3us DMA; top up with free transposes so the
                    # engine neither idles nor cools down
                    for _ in range(10):
                        nc.tensor.transpose(warm[:], ident[:], ident[:])

            def k_rope():
                pkv = st["p2"]
                # ACT pulls the PSUM slab to SBUF bf16 (unscaling sx*sw) so
                # every DVE op below runs in 2-byte all-SBUF 4x mode
                ks = work.tile([128, 512], BF16, tag="ks", name="ks")
                nc.scalar.activation(ks[:], pkv[:],
                                     mybir.ActivationFunctionType.Copy,
                                     scale=1.0 / (SX * SW))
                shufk = work.tile([64, 512], BF16, tag="shufk", name="shufk")
                m1k = work.tile([64, 512], BF16, tag="m1k", name="m1k")
                t2k = work.tile([64, 512], BF16, tag="t2k", name="t2k")
                kb = work.tile([64, 512], BF16, tag="kb", name="kb")
                nc.vector.stream_shuffle(shufk[:], ks[0:64, :],
                                         _SHUF_SWAP16)
                nc.vector.tensor_mul(m1k[:], ks[0:64, :],
                                     cos_sb[0:64, s0:s0 + 512])
                nc.vector.tensor_mul(t2k[:], shufk[:],
                                     sin_sb[0:64, s0:s0 + 512])
                nc.vector.tensor_add(kb[:], m1k[:], t2k[:])
                nc.vector.tensor_copy(K8[:, 0, s0:s0 + 512], kb[:])
                nc.vector.scalar_tensor_tensor(
                    K8[:, 1, s0:s0 + 512], K8[:, 0, s0:s0 + 512], -1.0,
                    kb[:], op0=MULT, op1=ADD)
                nc.vector.tensor_copy(Vt[64:128, s0:s0 + 512],
                                      ks[64:128, :])

            def v_trans(kt):
                pv = pw.tile([128, HD], BF16, tag="pw", name="pv")
                nc.tensor.transpose(
                    pv[:], Vt[64:128, kt * 128:(kt + 1) * 128],
                    ident[64:128, 64:128])
                nc.vector.tensor_copy(
                    Vp[:, kt * (HD + 1):kt * (HD + 1) + HD], pv[:])

            def q_mm(mt, kt):
                if kt == 0:
                    st[f"p{mt}"] = pjo.tile([128, 512], F32, tag="pjo",
                                            name="pq")
                if kt % 2 == 1:
                    for term in range(3):
                        drp(mt, kt // 2, term)

            def q_rope(mt):
                pq = st[f"p{mt}"]
                qs = work.tile([128, 512], BF16, tag="qs", name="qs")
                nc.scalar.activation(qs[:], pq[:],
                                     mybir.ActivationFunctionType.Copy,
                                     scale=1.0 / (SX * SW))
                shuf = work.tile([128, 512], BF16, tag="shuf", name="shuf")
                m1 = work.tile([128, 512], BF16, tag="m1", name="m1")
                t2 = work.tile([128, 512], BF16, tag="t2", name="t2")
                nc.vector.stream_shuffle(shuf[:], qs[:], _SHUF_SWAP16)
                nc.vector.tensor_mul(m1[:], qs[:], cos_sb[:, s0:s0 + 512])
                nc.vector.tensor_mul(t2[:], shuf[:], sin_sb[:, s0:s0 + 512])
                he = (2 * mt) * S
                ho = (2 * mt + 1) * S
                # fp8 write directly from the rope add (plain-e4m3 Q side)
                nc.vector.tensor_add(Q8[:, he + s0:he + s0 + 512],
                                     m1[0:64, :], t2[0:64, :])
                nc.vector.tensor_add(Q8[:, ho + s0:ho + s0 + 512],
                                     m1[64:128, :], t2[64:128, :])

            th = [lambda kt=kt: dma_kv(kt) for kt in range(KT_D)]
            if fused:
                th += [lambda: q_rope(0), k_rope, lambda: q_rope(1)]
                th += [lambda kt=kt: v_trans(kt)
                       for kt in range(4 * sc, 4 * sc + 4)]
                return th
            th.append(k_rope)
            th += [lambda kt=kt: v_trans(kt)
                   for kt in range(4 * sc, 4 * sc + 4)]
            if True:
                for mt in range(2):
                    th += [lambda mt=mt, kt=kt: q_mm(mt, kt)
                           for kt in range(KT_D)]
                    th.append(lambda mt=mt: q_rope(mt))
            return th

        def s_thunks(qc, h, tiles):
            """S^T DoubleRow matmuls + exp + mask for one head."""
            q0 = qc * 512
            hf = h * S
            nkt = 4 * qc + 4
            thunks = []
            for pi in range(nkt // 2):
                def th(pi=pi):
                    kts = (2 * pi, 2 * pi + 1)
                    ps_t = ps.tile([128, 1024], F32, tag="ps", name="ps_t")
                    pt_t = ptp.tile([128, 1024], BF16, tag="pt", name="pt_t")
                    for li, kt in enumerate(kts):
                        dj = kt - 4 * qc
                        qo = 128 * dj if dj >= 0 else 0
                        if 2 * pi == 4 * qc:
                            qo = 0  # keep the merged full-width exp legal
                        lo = li * 512
                        nc.tensor.matmul(
                            ps_t[:, lo + qo:lo + 512],
                            K8[:, :, kt * 128:(kt + 1) * 128],
                            Q8[:, hf + q0 + qo:hf + q0 + 512]
                            .unsqueeze(1).broadcast_to((64, 2, 512 - qo)),
                            start=True, stop=True, perf_mode=DR)
                    if 2 * pi + 1 < 4 * qc or 2 * pi == 4 * qc:
                        # interior pair, or the (dj0,dj1) boundary pair:
                        # one full-width exp (the dj1 junk region is never
                        # read downstream) — fewer ACT instructions
                        nc.scalar.activation(
                            pt_t[:], ps_t[:], EXP,
                            scale=SCALE / (SQ * SQ))
                    else:
                        for li, kt in enumerate(kts):
                            dj = kt - 4 * qc
                            qo = 128 * dj if dj >= 0 else 0
                            lo = li * 512
                            nc.scalar.activation(
                                pt_t[:, lo + qo:lo + 512],
                                ps_t[:, lo + qo:lo + 512], EXP,
                                scale=SCALE / (SQ * SQ))
                    for li, kt in enumerate(kts):
                        dj = kt - 4 * qc
                        qo = 128 * dj if dj >= 0 else 0
                        lo = li * 512
                        if dj >= 0:
                            # causal mask on the diagonal block: keep
                            # j >= p, zero below — affine predicate on the
                            # (otherwise idle) GPSIMD engine
                            nc.gpsimd.affine_select(
                                out=pt_t[:, lo + qo:lo + qo + 128],
                                in_=pt_t[:, lo + qo:lo + qo + 128],
                                compare_op=mybir.AluOpType.is_ge,
                                fill=0.0, base=0,
                                pattern=[[1, 128]],
                                channel_multiplier=-1)
                        tiles.append((kt, qo, lo, pt_t))
                thunks.append(th)
            return thunks

        def pv_thunks(qc, h, tiles):
            """PV accumulation pairs + final normalization for one head."""
            q0 = qc * 512
            hp = (h % 2) * 64
            nkt0 = 4 * qc + 4
            state = {}

            def pv_pair(pi):
                if "po" not in state:
                    state["po"] = pjo.tile([HD + 1, 512], F32, tag="pjo",
                                           name="pot")
                po_t = state["po"]
                for kt, qo, lo, pt_t in tiles[2 * pi:2 * pi + 2]:
                    nc.tensor.matmul(
                        po_t[:, qo:512],
                        Vp[:, kt * (HD + 1):(kt + 1) * (HD + 1)],
                        pt_t[:, lo + qo:lo + 512],
                        start=(kt == 0), stop=(kt == nkt0 - 1))
                if 2 * pi + 2 >= nkt0:
                    rc = small.tile([1, 512], F32, tag="rc", name="rc")
                    rb = small.tile([64, 512], F32, tag="rb", name="rb")
                    nc.vector.reciprocal(rc[:], po_t[64:65, :])
                    nc.gpsimd.partition_broadcast(rb[:], rc[:])
                    of = (h // 2) * S
                    nc.vector.tensor_mul(
                        OT[hp:hp + 64, of + q0:of + q0 + 512],
                        po_t[0:64, :], rb[:])

            return [lambda pi=pi: pv_pair(pi) for pi in range(nkt0 // 2)]

        def wo_half(qt, np2, half, obs, pool=None, ptag="pw",
                    act_copy=False, split_dma=False):
            """One 512-wide n-chunk; the second half fires the paired
            [128,1024] output DMA."""
            pool = pool or pw
            if half == 0:
                obs[(qt, np2)] = osb.tile([128, 1024], BF16, tag="ob",
                                          name="ob")
            ob = obs[(qt, np2)]
            ncn = 2 * np2 + half
            pw_t = pool.tile([128, 512], F32, tag=ptag, name="pw_t")
            for mt in range(2):
                nc.tensor.matmul(
                    pw_t[:],
                    OT[:, mt * S + qt * 128:mt * S + (qt + 1) * 128],
                    wo_sb[:, mt * DIM + ncn * 512:mt * DIM + ncn * 512 + 512],
                    start=(mt == 0), stop=(mt == 1))
            if act_copy:
                nc.scalar.copy(ob[:, half * 512:half * 512 + 512], pw_t[:])
            else:
                nc.vector.tensor_copy(
                    ob[:, half * 512:half * 512 + 512], pw_t[:])
            if split_dma:
                nc.sync.dma_start(
                    out[qt * 128:(qt + 1) * 128, ncn * 512:ncn * 512 + 512],
                    ob[:, half * 512:half * 512 + 512])
                if half == 1:
                    del obs[(qt, np2)]
                    wo_copy_flip[0] += 1
            elif half == 1:
                del obs[(qt, np2)]
                wo_copy_flip[0] += 1
                nc.sync.dma_start(
                    out[qt * 128:(qt + 1) * 128,
                        np2 * 1024:np2 * 1024 + 1024], ob[:])
        wo_obs = {}

        # ------------------------------------- merged emission schedule
        def merge(primary, *others):
            """Emit primary thunks; proportionally interleave the others."""
            counters = [0.0] * len(others)
            n = max(1, len(primary))
            for beat in primary:
                for j, lst in enumerate(others):
                    counters[j] += len(lst) / n
                    while counters[j] >= 1.0 and lst:
                        lst.pop(0)()
                        counters[j] -= 1.0
                for th in beat:
                    th()
            for lst in others:
                while lst:
                    lst.pop(0)()

        for th in proj_thunks(0, fused=True):       # prologue
            th()

        prev = None                      # (qc, h, tiles) awaiting PV
        for sc in range(NSC):
            if sc == 1:
                nc.sync.dma_start(wo_sb[:, 0:DIM], wo_s[0:128, :])
                nc.sync.dma_start(wo_sb[:, DIM:2 * DIM], wo_s[128:256, :])
            pstream = proj_thunks(sc + 1) if sc + 1 < NSC else []
            wostream = ([lambda qt=qt, np2=np2, half=half:
                         wo_half(qt, np2, half, wo_obs)
                         for qt in range(4 * (sc - 1), 4 * (sc - 1) + 4)
                         for np2 in range(2)
                         for half in range(2)] if sc >= 1 else [])
            for h in range(HQ):
                tiles = []
                sth = s_thunks(sc, h, tiles)
                pth = pv_thunks(*prev) if prev is not None else []
                beats = []
                for i in range(max(len(sth), len(pth))):
                    beat = []
                    if i < len(pth):
                        beat.append(pth[i])
                    if i < len(sth):
                        beat.append(sth[i])
                    beats.append(beat)
                if h == 0:
                    # front-load the next chunk's KV phase: the first score
                    # DRs wait on this chunk's Q8 (DVE rope chain), so give
                    # the PE independent work to chew first
                    ptake = min(len(pstream), 23)
                    wtake = (len(wostream) // HQ) if wostream else 0
                    merge(beats, pstream[:ptake], wostream[:wtake])
                    pstream = pstream[ptake:]
                    wostream = wostream[wtake:]
                else:
                    ptake = (len(pstream) // (HQ - h)) if pstream else 0
                    wtake = (len(wostream) // (HQ - h)) if wostream else 0
                    merge(beats, pstream[:ptake], wostream[:wtake])
                    pstream = pstream[ptake:]
                    wostream = wostream[wtake:]
                prev = (sc, h, tiles)
            merge([], pstream, wostream)

        # epilogue: PV of the last head, then WO of chunk 3
        for th in pv_thunks(*prev):
            th()
        epi = 0
        pools = [(pw, "pw"), (ps, "ps"), (pjo, "pjo")]
        for qt in range(12, 16):
            for np2 in range(2):
                pool, ptag = pools[epi % 3]
                for half in range(2):
                    wo_half(qt, np2, half, wo_obs, pool=pool, ptag=ptag,
                            act_copy=(half == 1), split_dma=True)
                epi += 1

    nc.compile()
    return nc


# ------------------------------------------------------------- host side
def _pair_perm64():
    """Column permutation putting the RoPE partner 16 partitions away."""
    return np.array([2 * (16 * (j // 32) + (j % 16)) + ((j % 32) // 16)
                     for j in range(64)])


def _f8(a):
    return np.ascontiguousarray(a.astype(ml_dtypes.float8_e4m3fn))


def _host_prep(x, freqs_cos, freqs_sin, wq, wk, wv, wo):
    x = np.asarray(x, np.float32)
    fc = np.asarray(freqs_cos, np.float32)
    fs = np.asarray(freqs_sin, np.float32)
    wq = np.asarray(wq, np.float32)
    wk = np.asarray(wk, np.float32)
    wv = np.asarray(wv, np.float32)
    wo = np.asarray(wo, np.float32)

    perm = _pair_perm64()
    xT = np.ascontiguousarray(x[0].T) * SX
    xT8 = _f8(xT)
    xTlo = _f8(xT - xT8.astype(np.float32))

    p = np.arange(64)
    pair = 16 * ((p % 64) // 32) + (p % 16)
    sign = np.where((p % 32) < 16, -1.0, 1.0).astype(np.float32)
    fold = SQ
    cosE = (np.ascontiguousarray(fc[:, pair].T) * fold).astype(
        ml_dtypes.bfloat16)                                      # [64, S]
    sinE = (np.ascontiguousarray(fs[:, pair].T) * sign[:, None] * fold
            ).astype(ml_dtypes.bfloat16)

    in_maps = []
    for c in range(NCORES):
        qcols = np.concatenate(
            [wq[:, (4 * c + i) * 64 + perm] for i in range(HQ)], axis=1)
        kcols = wk[:, c * 64 + perm]
        vcols = wv[:, c * 64:(c + 1) * 64]
        wqkv_c = np.concatenate([qcols, kcols, vcols], axis=1) * SW
        wqkv8 = _f8(wqkv_c)
        wqkvlo = _f8(wqkv_c - wqkv8.astype(np.float32))
        wo_c = wo[QW * c:QW * (c + 1), :].astype(ml_dtypes.bfloat16)
        in_maps.append({
            "xT8": xT8,
            "xTlo": xTlo,
            "wqkv8": wqkv8,
            "wqkvlo": wqkvlo,
            "wo_s": np.ascontiguousarray(wo_c),
            "cosE": cosE,
            "sinE": np.ascontiguousarray(sinE),
        })
    return in_maps


_NC_CACHE = {}


def get_program():
    if "v2" not in _NC_CACHE:
        _NC_CACHE["v2"] = build_program()
    return _NC_CACHE["v2"]


def kernel(x, freqs_cos, freqs_sin, wq, wk, wv, wo):
    nc = get_program()
    in_maps = _host_prep(x, freqs_cos, freqs_sin, wq, wk, wv, wo)
    res = run_bass_kernel_spmd(nc, in_maps, core_ids=list(range(NCORES)))
    acc = np.zeros((S, DIM), np.float32)
    for r in res.results:
        acc += np.asarray(r["out"], dtype=np.float32)
    return acc.reshape(1, S, DIM)
